# revision 1
# baseline (speedup 1.0000x reference)
"""CodeGen attention block (B=1, S=2048, E=2048, H=16, D=128, rot=64) on 8 TRN2
NeuronCores.

Sharding: tensor-parallel over heads (2 heads/core). Each core computes its
heads' q/k in transposed [d, s] layout (partial rotary applied via a host-side
even/odd channel permutation folded into the qkv weights, which turns the
interleaved rotation into a rotate-half form on contiguous partition blocks),
v in [s, d] layout, causal softmax attention entirely on-chip (scores
transposed [k, q] so the P·V matmul needs no transposes; softmax denominators
via a ones-stationary matmul), then AllGathers the per-core attention output
O^T [256, 2048] and computes a 256-column slice of the output projection.
Host assembles the 8 slices and transposes.

All PE-facing tensors are bf16 (full PE speed, half the DMA); accumulation
is fp32 in PSUM. The AllGather is split into 4 per-s-chunk collectives that
fire as soon as both heads finish a chunk, overlapping comm with attention.
"""

import numpy as np

H, D, ROT, MP = 16, 128, 64, 4
S, E = 2048, 2048
NCORES = 8
P = 128
NQ = 4            # 512-wide q chunks
NKT = S // P      # 16 k tiles
NEC = E // P      # 16 contraction chunks
SCALE = float(1.0 / np.sqrt(np.float64(D)))

_CACHE = {}


# ----------------------------------------------------------------------------
# host-side input prep
# ----------------------------------------------------------------------------

def _head_rows(h):
    g, j = h // 4, h % 4
    base = g * (3 * 512)
    q = np.arange(base + j * 128, base + (j + 1) * 128)
    v = np.arange(base + 512 + j * 128, base + 512 + (j + 1) * 128)
    k = np.arange(base + 1024 + j * 128, base + 1024 + (j + 1) * 128)
    return q, v, k


def _prep_core_weights(c, w_qkv, w_out):
    h0, h1 = 2 * c, 2 * c + 1
    top = np.arange(0, ROT, 2)
    bot = np.arange(1, ROT, 2)
    wq, wk, wv = {}, {}, {}
    for h in (h0, h1):
        qr, vr, kr = _head_rows(h)
        wq[h], wv[h], wk[h] = w_qkv[qr], w_qkv[vr], w_qkv[kr]
    G0 = np.concatenate([wq[h0][top], wq[h1][top], wk[h0][top], wk[h1][top]], 0)
    G1 = np.concatenate([wq[h0][bot], wq[h1][bot], wk[h0][bot], wk[h1][bot]], 0)
    G2 = np.concatenate([wq[h0][ROT:], wq[h1][ROT:]], 0)
    G3 = np.concatenate([wk[h0][ROT:], wk[h1][ROT:]], 0)
    wqkT = np.ascontiguousarray(
        np.concatenate([G0, G1, G2, G3], 0).T, dtype=np.float32)       # [E, 512]
    wvT = np.ascontiguousarray(
        np.concatenate([wv[h0], wv[h1]], 0).T, dtype=np.float32)       # [E, 256]
    woutT = np.ascontiguousarray(
        w_out[256 * c:256 * (c + 1), :].T, dtype=np.float32)           # [E, 256]
    # pre-pack for a fully contiguous [128, 16, 256] SBUF load
    woutT = np.ascontiguousarray(
        woutT.reshape(16, 128, 256).transpose(1, 0, 2)).reshape(128, 16 * 256)
    return wqkT, wvT, woutT


def _cos_sin():
    inv_freq = 1.0 / (10000.0 ** (np.arange(0, ROT, 2, dtype=np.float32) / ROT))
    ang = np.arange(S, dtype=np.float32)[:, None] * inv_freq[None, :]
    cosb = np.cos(ang).T.astype(np.float32)        # [32, S]
    sinb = np.sin(ang).T.astype(np.float32)
    return (np.ascontiguousarray(np.tile(cosb, (4, 1))),
            np.ascontiguousarray(np.tile(sinb, (4, 1))))               # [128, S]


def _mask_tiles():
    m = np.zeros((128, 4 * 512), dtype=np.float32)
    pp = np.arange(128)[:, None]
    cc = np.arange(512)[None, :]
    for mm in range(4):
        m[:, mm * 512:(mm + 1) * 512] = ((pp + 128 * mm) <= cc).astype(np.float32)
    return m


def _build_in_maps(hidden_states, w_qkv, w_out):
    import ml_dtypes
    bf16 = ml_dtypes.bfloat16
    hiddenT = np.ascontiguousarray(
        np.asarray(hidden_states, np.float32).reshape(S, E).T).astype(bf16)
    COS, SIN = _cos_sin()
    COS, SIN = COS.astype(bf16), SIN.astype(bf16)
    masks = _mask_tiles().astype(bf16)
    in_maps = []
    for c in range(NCORES):
        wqkT, wvT, woutT = _prep_core_weights(
            c, np.asarray(w_qkv, np.float32), np.asarray(w_out, np.float32))
        in_maps.append({
            "hiddenT": hiddenT,
            "wqkT": wqkT.astype(bf16),
            "wvT": wvT.astype(bf16),
            "woutT": woutT.astype(bf16),
            "cosT": COS,
            "sinT": SIN,
            "masks": masks,
        })
    return in_maps


# ----------------------------------------------------------------------------
# device program
# ----------------------------------------------------------------------------

def _kernel_body(tc, outT, hiddenT, wqkT, wvT, woutT, cosT, sinT, masksT):
    import concourse.mybir as mybir
    from contextlib import ExitStack

    nc = tc.nc
    f32 = mybir.dt.float32
    bt = mybir.dt.bfloat16

    def r(ap):
        return ap

    with ExitStack() as ctx:
        const = ctx.enter_context(tc.tile_pool(name="const", bufs=1))
        mask_sb = const.tile([P, 4 * 512], bt, name="mask_sb")
        ones_sb = const.tile([P, P], bt, name="ones_sb")
        wo_sb = const.tile([P, NEC, 256], bt, name="wo_sb")
        nc.gpsimd.dma_start(mask_sb[:], masksT)
        nc.gpsimd.dma_start(wo_sb[:], woutT.rearrange("p (o e) -> p o e", o=NEC))
        nc.vector.memset(ones_sb[:], 1.0)

        dram = ctx.enter_context(tc.tile_pool(name="dram", bufs=1, space="DRAM"))
        ag_in = [dram.tile([2 * P, 512], bt, name=f"ag_in{j}", tag=f"ag_in{j}")
                 for j in range(NQ)]
        ag_out = [dram.tile([E, 512], bt, name=f"ag_out{j}", tag=f"ag_out{j}",
                            addr_space="Shared") for j in range(NQ)]

        # --- phase 1+2 interleaved: per-chunk qkv -> attention -> AG --------
        with tc.tile_pool(name="ph1c", bufs=1) as ph1c_pool, \
             tc.tile_pool(name="chunk", bufs=1) as ck_pool, \
             tc.tile_pool(name="hid", bufs=2 * NEC) as hid_pool, \
             tc.tile_pool(name="wts", bufs=NEC) as wts_pool, \
             tc.tile_pool(name="g01c", bufs=2) as g01_pool, \
             tc.tile_pool(name="rtmp", bufs=2) as rtmp_pool, \
             tc.tile_pool(name="pt", bufs=6) as pt_pool, \
             tc.tile_pool(name="oout", bufs=3) as oo_pool, \
             tc.tile_pool(name="otin", bufs=2) as oi_pool, \
             tc.tile_pool(name="oprj", bufs=3) as op_pool, \
             tc.tile_pool(name="gps", bufs=2, space="PSUM") as gps_pool, \
             tc.tile_pool(name="scps", bufs=2, space="PSUM") as sc_pool, \
             tc.tile_pool(name="otps", bufs=1, space="PSUM") as ot_pool, \
             tc.tile_pool(name="dnps", bufs=1, space="PSUM") as dn_pool, \
             tc.tile_pool(name="opps", bufs=2, space="PSUM") as pp_pool:

            cos_sb = ph1c_pool.tile([P, S], bt, name="cos_sb", tag="cos_sb")
            sin_sb = ph1c_pool.tile([P, S], bt, name="sin_sb", tag="sin_sb")
            nc.gpsimd.dma_start(cos_sb[:], cosT)
            nc.gpsimd.dma_start(sin_sb[:], sinT)

            # per-chunk activations: q/k transposed [d, 512]; v [k-in-tile, 4*128]
            qc = [[ck_pool.tile([P, 512], bt, name=f"qc{h}_{j}", tag=f"qc{h}_{j}")
                   for j in range(NQ)] for h in range(2)]
            kc = [[ck_pool.tile([P, 512], bt, name=f"kc{h}_{j}", tag=f"kc{h}_{j}")
                   for j in range(NQ)] for h in range(2)]
            vc = [[ck_pool.tile([P, 512], bt, name=f"vc{h}_{j}", tag=f"vc{h}_{j}")
                   for j in range(NQ)] for h in range(2)]

            wqk_sb = []
            wv_sb = []
            hid_sb = []  # per e: (lo, hi) halves of the hiddenT row-chunk
            for e in range(NEC):
                wq_tile = wts_pool.tile([P, 512], bt, name=f"wqk_{e}", tag="wqk",
                                        bufs=NEC)
                nc.gpsimd.dma_start(wq_tile[:], wqkT[e * P:(e + 1) * P, :])
                wqk_sb.append(wq_tile)
                wv_tile = wts_pool.tile([P, 256], bt, name=f"wv_{e}", tag="wv",
                                        bufs=NEC)
                nc.gpsimd.dma_start(wv_tile[:], wvT[e * P:(e + 1) * P, :])
                wv_sb.append(wv_tile)
                halves = []
                for hh in range(2):
                    ht = hid_pool.tile([P, 1024], bt, name=f"hid_{e}_{hh}",
                                       tag="hid", bufs=2 * NEC)
                    heng = nc.sync if e % 2 == 0 else nc.scalar
                    heng.dma_start(
                        ht[:], hiddenT[e * P:(e + 1) * P,
                                       1024 * hh:1024 * (hh + 1)])
                    halves.append(ht)
                hid_sb.append(halves)

            oti_sb = []

            def hidsl(e, lo, width):  # [lo, lo+width) columns of chunk e
                half = hid_sb[e][lo // 1024]
                off = lo % 1024
                return half[:, off:off + width]

            for j in range(NQ):
                qs = slice(512 * j, 512 * (j + 1))
                # ---- qkv chunk j: rot groups in 2 waves of 2 psum banks ----
                g01 = []
                for g in (0, 1):
                    gp = gps_pool.tile([P, 512], f32, name=f"gps{j}_{g}",
                                       tag="gps")
                    for e in range(NEC):
                        nc.tensor.matmul(
                            gp[:], wqk_sb[e][:, g * P:(g + 1) * P],
                            hidsl(e, 512 * j, 512), start=(e == 0),
                            stop=(e == NEC - 1))
                    gc = g01_pool.tile([P, 512], bt, name=f"g01_{j}_{g}",
                                       tag=f"g01_{g}")
                    nc.scalar.copy(gc[:], gp[:])
                    g01.append(gc)
                for g in (2, 3):
                    gp = gps_pool.tile([P, 512], f32, name=f"gps{j}_{g}",
                                       tag="gps")
                    for e in range(NEC):
                        nc.tensor.matmul(
                            gp[:], wqk_sb[e][:, g * P:(g + 1) * P],
                            hidsl(e, 512 * j, 512), start=(e == 0),
                            stop=(e == NEC - 1))
                    dst = qc if g == 2 else kc
                    nc.vector.tensor_copy(dst[0][j][64:128, :], gp[0:64, :])
                    nc.vector.tensor_copy(dst[1][j][64:128, :], gp[64:128, :])
                # v chunk j
                for st in range(4):
                    vp = gps_pool.tile([P, 256], f32, name=f"vps{j}_{st}",
                                       tag="gps")
                    for e in range(NEC):
                        nc.tensor.matmul(
                            vp[:], hidsl(e, 512 * j + st * P, P),
                            wv_sb[e][:], start=(e == 0), stop=(e == NEC - 1))
                    nc.vector.tensor_copy(vc[0][j][:, st * P:(st + 1) * P],
                                          vp[:, 0:P])
                    nc.vector.tensor_copy(vc[1][j][:, st * P:(st + 1) * P],
                                          vp[:, P:2 * P])
                # rope chunk j
                t0 = rtmp_pool.tile([P, 512], bt, name=f"t0_{j}", tag="t0")
                t1 = rtmp_pool.tile([P, 512], bt, name=f"t1_{j}", tag="t1")
                ta = rtmp_pool.tile([P, 512], bt, name=f"ta_{j}", tag="ta")
                tb = rtmp_pool.tile([P, 512], bt, name=f"tb_{j}", tag="tb")
                nc.vector.tensor_mul(t0[:], g01[0][:], cos_sb[:, qs])
                nc.vector.tensor_mul(t1[:], g01[1][:], sin_sb[:, qs])
                nc.vector.tensor_sub(ta[:], t0[:], t1[:])      # tops
                nc.vector.tensor_mul(t0[:], g01[1][:], cos_sb[:, qs])
                nc.vector.tensor_mul(t1[:], g01[0][:], sin_sb[:, qs])
                nc.vector.tensor_add(tb[:], t0[:], t1[:])      # bottoms
                for pc, dst in enumerate((qc[0], qc[1], kc[0], kc[1])):
                    ps = slice(32 * pc, 32 * (pc + 1))
                    nc.vector.tensor_copy(dst[j][0:32, :], ta[ps, :])
                    nc.vector.tensor_copy(dst[j][32:64, :], tb[ps, :])

                # ---- attention chunk j (both heads) ----
                nk = 4 * j + 4
                for hi in range(2):
                    otp = ot_pool.tile([P, 512], f32, name=f"otp{hi}_{j}",
                                       tag="otp")
                    dnp = dn_pool.tile([P, 512], f32, name=f"dnp{hi}_{j}",
                                       tag="dnp")
                    for i in range(nk):
                        scp = sc_pool.tile([P, 512], f32, name=f"scp{hi}{j}{i}",
                                           tag="scp")
                        nc.tensor.matmul(
                            scp[:], kc[hi][i // 4][:, (i % 4) * P:
                                                   (i % 4 + 1) * P],
                            qc[hi][j][:], start=True, stop=True)
                        pt = pt_pool.tile([P, 512], bt, name=f"pt{hi}{j}{i}",
                                          tag="pt")
                        nc.scalar.activation(
                            pt[:], scp[:], mybir.ActivationFunctionType.Exp,
                            scale=SCALE)
                        if i >= 4 * j:
                            mm = i - 4 * j
                            nc.vector.tensor_mul(
                                pt[:], pt[:],
                                mask_sb[:, mm * 512:(mm + 1) * 512])
                        nc.tensor.matmul(
                            otp[:], vc[hi][i // 4][:, (i % 4) * P:
                                                   (i % 4 + 1) * P], pt[:],
                            start=(i == 0), stop=(i == nk - 1))
                        nc.tensor.matmul(
                            dnp[:], ones_sb[:], pt[:],
                            start=(i == 0), stop=(i == nk - 1))
                    den_sb = oo_pool.tile([P, 512], f32, name=f"den{hi}_{j}",
                                          tag="den")
                    nc.vector.reciprocal_approx_fast(den_sb[:], dnp[:])
                    otn = oo_pool.tile([P, 512], bt, name=f"otn{hi}_{j}",
                                       tag="otn")
                    nc.vector.tensor_mul(otn[:], otp[:], den_sb[:])
                    nc.sync.dma_start(ag_in[j][hi * P:(hi + 1) * P, :], otn[:])
                nc.gpsimd.collective_compute(
                    "AllGather",
                    mybir.AluOpType.bypass,
                    replica_groups=[list(range(NCORES))],
                    ins=[ag_in[j].opt()],
                    outs=[ag_out[j].opt()],
                )
                # prefetch this chunk's gathered O^T as one contiguous 2MB load
                oti = oi_pool.tile([P, NEC, 512], bt, name=f"oti{j}", tag="oti")
                nc.scalar.dma_start(
                    oti[:], ag_out[j].rearrange("(o p) s -> p o s", p=P))
                oti_sb.append(oti)

            # ---- output projection (PE order: after all attention) ----
            for j in range(NQ):
                qs = slice(512 * j, 512 * (j + 1))
                pps = [pp_pool.tile([P, 512], f32, name=f"pps{j}_{b}", tag="pps")
                       for b in range(2)]
                for fc in range(NEC):
                    for b in range(2):
                        nc.tensor.matmul(
                            pps[b][:], wo_sb[:, fc, b * P:(b + 1) * P],
                            oti_sb[j][:, fc, :], start=(fc == 0),
                            stop=(fc == NEC - 1))
                for b in range(2):
                    ob = op_pool.tile([P, 512], f32, name=f"ob{j}_{b}", tag="ob")
                    nc.scalar.copy(ob[:], pps[b][:])
                    nc.sync.dma_start(outT[b * P:(b + 1) * P, qs], ob[:])


def _build_program():
    import concourse.bass as bass  # noqa: F401
    import concourse.mybir as mybir
    import concourse.tile as tile
    from concourse import bacc

    nc = bacc.Bacc("TRN2", target_bir_lowering=False, debug=False,
                   enable_asserts=False, num_devices=NCORES)
    f32 = mybir.dt.float32
    bt = mybir.dt.bfloat16
    hiddenT = nc.dram_tensor("hiddenT", [E, S], bt, kind="ExternalInput").ap()
    wqkT = nc.dram_tensor("wqkT", [E, 512], bt, kind="ExternalInput").ap()
    wvT = nc.dram_tensor("wvT", [E, 256], bt, kind="ExternalInput").ap()
    woutT = nc.dram_tensor("woutT", [P, NEC * 256], bt, kind="ExternalInput").ap()
    cosT = nc.dram_tensor("cosT", [P, S], bt, kind="ExternalInput").ap()
    sinT = nc.dram_tensor("sinT", [P, S], bt, kind="ExternalInput").ap()
    masks = nc.dram_tensor("masks", [P, 4 * 512], bt, kind="ExternalInput").ap()
    outT = nc.dram_tensor("outT", [2 * P, S], f32, kind="ExternalOutput").ap()

    with tile.TileContext(nc) as tc:
        _kernel_body(tc, outT, hiddenT, wqkT, wvT, woutT, cosT, sinT, masks)
    nc.compile()
    return nc


def get_program():
    if "nc" not in _CACHE:
        _CACHE["nc"] = _build_program()
    return _CACHE["nc"]


def _install_ntff_shim():
    """Provide antenv.axon_hooks (missing in this image) so trace=True works."""
    import sys
    import types
    try:
        import antenv.axon_hooks  # noqa: F401
        return
    except ImportError:
        pass
    import antenv
    mod = types.ModuleType("antenv.axon_hooks")
    mod._hook = None

    def set_axon_ntff_profile_hook(h):
        mod._hook = h

    def get_axon_ntff_profile_hook():
        return mod._hook

    mod.set_axon_ntff_profile_hook = set_axon_ntff_profile_hook
    mod.get_axon_ntff_profile_hook = get_axon_ntff_profile_hook
    sys.modules["antenv.axon_hooks"] = mod
    antenv.axon_hooks = mod
    try:
        from trn_agent_boot.trn_boot import _ntff_profile_via_ctypes
        hook = _ntff_profile_via_ctypes("/opt/axon/libaxon_pjrt.so")
        if hook is not None:
            mod._hook = hook
    except Exception:
        pass


def run(inputs, trace=False):
    """Run on the 8 NeuronCores; returns (out [1,S,E], BassKernelResults)."""
    from concourse import bass_utils

    if trace:
        _install_ntff_shim()
    nc = get_program()
    in_maps = _build_in_maps(inputs["hidden_states"], inputs["w_qkv"],
                             inputs["w_out"])
    res = bass_utils.run_bass_kernel_spmd(
        nc, in_maps, core_ids=list(range(NCORES)), trace=trace)
    outT = np.concatenate([res.results[c]["outT"] for c in range(NCORES)],
                          axis=0)  # [E, S]
    out = np.ascontiguousarray(outT.T).reshape(1, S, E).astype(np.float32)
    return out, res


def kernel(hidden_states, w_qkv, w_out):
    out, _ = run({"hidden_states": hidden_states, "w_qkv": w_qkv,
                  "w_out": w_out})
    return out



# revision 2
# speedup vs baseline: 1.2546x; 1.2546x over previous
"""CodeGen attention block (B=1, S=2048, E=2048, H=16, D=128, rot=64) on 8 TRN2
NeuronCores.

Sharding: tensor-parallel over heads (2 heads/core). Each core computes its
heads' q/k in transposed [d, s] layout (partial rotary applied via a host-side
even/odd channel permutation folded into the qkv weights, which turns the
interleaved rotation into a rotate-half form on contiguous partition blocks),
v in [s, d] layout, causal softmax attention entirely on-chip (scores
transposed [k, q] so the P·V matmul needs no transposes; softmax denominators
accumulated on the vector engine with a single ones-matmul partition-reduce),
then AllGathers the per-core attention output O^T [256, 2048] and computes a
256-column slice of the output projection. Host assembles the 8 slices.

All PE-facing tensors are bf16 (full PE speed, half the DMA); accumulation
is fp32 in PSUM. The AllGather is split into 4 per-s-chunk collectives that
fire as soon as both heads finish a chunk, overlapping comm with attention.
Diagonal score tiles are column-sliced to skip fully-masked work.
"""

import numpy as np

H, D, ROT, MP = 16, 128, 64, 4
S, E = 2048, 2048
NCORES = 8
P = 128
NQ = 4            # 512-wide q chunks
NKT = S // P      # 16 k tiles
NEC = E // P      # 16 contraction chunks
SCALE = float(1.0 / np.sqrt(np.float64(D)))

_CACHE = {}


# ----------------------------------------------------------------------------
# host-side input prep
# ----------------------------------------------------------------------------

def _head_rows(h):
    g, j = h // 4, h % 4
    base = g * (3 * 512)
    q = np.arange(base + j * 128, base + (j + 1) * 128)
    v = np.arange(base + 512 + j * 128, base + 512 + (j + 1) * 128)
    k = np.arange(base + 1024 + j * 128, base + 1024 + (j + 1) * 128)
    return q, v, k


def _prep_core_weights(c, w_qkv, w_out):
    h0, h1 = 2 * c, 2 * c + 1
    top = np.arange(0, ROT, 2)
    bot = np.arange(1, ROT, 2)
    wq, wk, wv = {}, {}, {}
    for h in (h0, h1):
        qr, vr, kr = _head_rows(h)
        wq[h], wv[h], wk[h] = w_qkv[qr], w_qkv[vr], w_qkv[kr]
    G0 = np.concatenate([wq[h0][top], wq[h1][top], wk[h0][top], wk[h1][top]], 0)
    G1 = np.concatenate([wq[h0][bot], wq[h1][bot], wk[h0][bot], wk[h1][bot]], 0)
    G2 = np.concatenate([wq[h0][ROT:], wq[h1][ROT:]], 0)
    G3 = np.concatenate([wk[h0][ROT:], wk[h1][ROT:]], 0)
    wqkT = np.ascontiguousarray(
        np.concatenate([G0, G1, G2, G3], 0).T, dtype=np.float32)       # [E, 512]
    wvT = np.ascontiguousarray(
        np.concatenate([wv[h0], wv[h1]], 0).T, dtype=np.float32)       # [E, 256]
    woutT = np.ascontiguousarray(
        w_out[256 * c:256 * (c + 1), :].T, dtype=np.float32)           # [E, 256]
    # pre-pack for a fully contiguous [128, 16, 256] SBUF load
    woutT = np.ascontiguousarray(
        woutT.reshape(16, 128, 256).transpose(1, 0, 2)).reshape(128, 16 * 256)
    return wqkT, wvT, woutT


def _cos_sin():
    inv_freq = 1.0 / (10000.0 ** (np.arange(0, ROT, 2, dtype=np.float32) / ROT))
    ang = np.arange(S, dtype=np.float32)[:, None] * inv_freq[None, :]
    cosb = np.cos(ang).T.astype(np.float32)        # [32, S]
    sinb = np.sin(ang).T.astype(np.float32)
    return (np.ascontiguousarray(np.tile(cosb, (4, 1))),
            np.ascontiguousarray(np.tile(sinb, (4, 1))))               # [128, S]


def _mask_tiles():
    m = np.zeros((128, 4 * 512), dtype=np.float32)
    pp = np.arange(128)[:, None]
    cc = np.arange(512)[None, :]
    for mm in range(4):
        m[:, mm * 512:(mm + 1) * 512] = ((pp + 128 * mm) <= cc).astype(np.float32)
    return m


def _build_in_maps(hidden_states, w_qkv, w_out):
    import ml_dtypes
    bf16 = ml_dtypes.bfloat16
    hiddenT = np.ascontiguousarray(
        np.asarray(hidden_states, np.float32).reshape(S, E).T).astype(bf16)
    COS, SIN = _cos_sin()
    COS, SIN = COS.astype(bf16), SIN.astype(bf16)
    masks = _mask_tiles().astype(bf16)
    in_maps = []
    for c in range(NCORES):
        wqkT, wvT, woutT = _prep_core_weights(
            c, np.asarray(w_qkv, np.float32), np.asarray(w_out, np.float32))
        in_maps.append({
            "hiddenT": hiddenT,
            "wqkT": wqkT.astype(bf16),
            "wvT": wvT.astype(bf16),
            "woutT": woutT.astype(bf16),
            "cosT": COS,
            "sinT": SIN,
            "masks": masks,
        })
    return in_maps


# ----------------------------------------------------------------------------
# device program
# ----------------------------------------------------------------------------

def _kernel_body(tc, outT, hiddenT, wqkT, wvT, woutT, cosT, sinT, masksT):
    import concourse.mybir as mybir
    from contextlib import ExitStack

    nc = tc.nc
    f32 = mybir.dt.float32
    bt = mybir.dt.bfloat16

    with ExitStack() as ctx:
        const = ctx.enter_context(tc.tile_pool(name="const", bufs=1))
        mask_sb = const.tile([P, 4 * 512], bt, name="mask_sb")
        ones_sb = const.tile([P, P], bt, name="ones_sb")
        wo_sb = const.tile([P, NEC, 256], bt, name="wo_sb")
        nc.vector.memset(ones_sb[:], 1.0)

        dram = ctx.enter_context(tc.tile_pool(name="dram", bufs=1, space="DRAM"))
        ag_in = [dram.tile([2 * P, 512], bt, name=f"ag_in{j}", tag=f"ag_in{j}")
                 for j in range(NQ)]
        ag_out = [dram.tile([E, 512], bt, name=f"ag_out{j}", tag=f"ag_out{j}",
                            addr_space="Shared") for j in range(NQ)]

        # --- phase 1+2 interleaved: per-chunk qkv -> attention -> AG --------
        with tc.tile_pool(name="ph1c", bufs=1) as ph1c_pool, \
             tc.tile_pool(name="chunk", bufs=1) as ck_pool, \
             tc.tile_pool(name="hid", bufs=2 * NEC) as hid_pool, \
             tc.tile_pool(name="wts", bufs=NEC) as wts_pool, \
             tc.tile_pool(name="g01c", bufs=2) as g01_pool, \
             tc.tile_pool(name="rtmp", bufs=2) as rtmp_pool, \
             tc.tile_pool(name="pt", bufs=4) as pt_pool, \
             tc.tile_pool(name="dac", bufs=1) as dac_pool, \
             tc.tile_pool(name="dacb", bufs=2) as dacb_pool, \
             tc.tile_pool(name="oout", bufs=2) as oo_pool, \
             tc.tile_pool(name="otin", bufs=2) as oi_pool, \
             tc.tile_pool(name="oprj", bufs=2) as op_pool, \
             tc.tile_pool(name="gps", bufs=2, space="PSUM") as gps_pool, \
             tc.tile_pool(name="scps", bufs=2, space="PSUM") as sc_pool, \
             tc.tile_pool(name="otps", bufs=2, space="PSUM") as ot_pool, \
             tc.tile_pool(name="opps", bufs=2, space="PSUM") as pp_pool:

            # --- initial loads: weights + hidden half0 first (qkv j=0,1),
            # hidden half1 second, constants on gpsimd in parallel ----------
            wqk_sb = []
            wv_sb = []
            hid_sb = []  # per e: [lo, hi] halves of the hiddenT row-chunk
            for e in range(NEC):
                eng_a = nc.sync if e % 2 == 0 else nc.scalar
                eng_b = nc.scalar if e % 2 == 0 else nc.sync
                wq_tile = wts_pool.tile([P, 512], bt, name=f"wqk_{e}", tag="wqk",
                                        bufs=NEC)
                eng_a.dma_start(wq_tile[:], wqkT[e * P:(e + 1) * P, :])
                wqk_sb.append(wq_tile)
                wv_tile = wts_pool.tile([P, 256], bt, name=f"wv_{e}", tag="wv",
                                        bufs=NEC)
                eng_b.dma_start(wv_tile[:], wvT[e * P:(e + 1) * P, :])
                wv_sb.append(wv_tile)
                ht0 = hid_pool.tile([P, 1024], bt, name=f"hid_{e}_0",
                                    tag="hid", bufs=2 * NEC)
                eng_b.dma_start(ht0[:], hiddenT[e * P:(e + 1) * P, 0:1024])
                hid_sb.append([ht0, None])
            for e in range(NEC):
                eng = nc.sync if e % 2 == 0 else nc.scalar
                ht1 = hid_pool.tile([P, 1024], bt, name=f"hid_{e}_1",
                                    tag="hid", bufs=2 * NEC)
                eng.dma_start(ht1[:], hiddenT[e * P:(e + 1) * P, 1024:2048])
                hid_sb[e][1] = ht1

            cos_sb = ph1c_pool.tile([P, S], bt, name="cos_sb", tag="cos_sb")
            sin_sb = ph1c_pool.tile([P, S], bt, name="sin_sb", tag="sin_sb")
            nc.gpsimd.dma_start(cos_sb[:], cosT)
            nc.gpsimd.dma_start(sin_sb[:], sinT)
            nc.gpsimd.dma_start(mask_sb[:], masksT)
            nc.gpsimd.dma_start(wo_sb[:], woutT.rearrange("p (o e) -> p o e",
                                                          o=NEC))

            # per-chunk activations: q/k transposed [d, 512]; v [k-in-tile, 4*128]
            qc = [[ck_pool.tile([P, 512], bt, name=f"qc{h}_{j}", tag=f"qc{h}_{j}")
                   for j in range(NQ)] for h in range(2)]
            kc = [[ck_pool.tile([P, 512], bt, name=f"kc{h}_{j}", tag=f"kc{h}_{j}")
                   for j in range(NQ)] for h in range(2)]
            vc = [[ck_pool.tile([P, 512], bt, name=f"vc{h}_{j}", tag=f"vc{h}_{j}")
                   for j in range(NQ)] for h in range(2)]

            oti_sb = {}

            def hidsl(e, lo, width):  # [lo, lo+width) columns of chunk e
                half = hid_sb[e][lo // 1024]
                off = lo % 1024
                return half[:, off:off + width]

            def emit_oti_load(j):
                # gathered O^T chunk as [128, 16, 512]; 8 sub-DMAs so the
                # transfer spreads across hardware DMA queues
                oti = oi_pool.tile([P, NEC, 512], bt, name=f"oti{j}", tag="oti")
                src = ag_out[j].rearrange("(o p) s -> p o s", p=P)
                for t in range(8):
                    nc.scalar.dma_start(oti[:, 2 * t:2 * t + 2, :],
                                        src[:, 2 * t:2 * t + 2, :])
                oti_sb[j] = oti

            def emit_oproj(j):
                qs = slice(512 * j, 512 * (j + 1))
                pps = [pp_pool.tile([P, 512], f32, name=f"pps{j}_{b}", tag="pps")
                       for b in range(2)]
                for fc in range(NEC):
                    for b in range(2):
                        nc.tensor.matmul(
                            pps[b][:], wo_sb[:, fc, b * P:(b + 1) * P],
                            oti_sb[j][:, fc, :], start=(fc == 0),
                            stop=(fc == NEC - 1))
                for b in range(2):
                    ob = op_pool.tile([P, 512], f32, name=f"ob{j}_{b}", tag="ob")
                    nc.vector.tensor_copy(ob[:], pps[b][:])
                    nc.sync.dma_start(outT[b * P:(b + 1) * P, qs], ob[:])

            for j in range(NQ):
                qs = slice(512 * j, 512 * (j + 1))
                # ---- qkv chunk j: rot groups in 2 waves of 2 psum banks ----
                g01 = []
                for g in (0, 1):
                    gp = gps_pool.tile([P, 512], f32, name=f"gps{j}_{g}",
                                       tag="gps")
                    for e in range(NEC):
                        nc.tensor.matmul(
                            gp[:], wqk_sb[e][:, g * P:(g + 1) * P],
                            hidsl(e, 512 * j, 512), start=(e == 0),
                            stop=(e == NEC - 1))
                    gc = g01_pool.tile([P, 512], bt, name=f"g01_{j}_{g}",
                                       tag=f"g01_{g}")
                    nc.scalar.copy(gc[:], gp[:])
                    g01.append(gc)
                for g in (2, 3):
                    gp = gps_pool.tile([P, 512], f32, name=f"gps{j}_{g}",
                                       tag="gps")
                    for e in range(NEC):
                        nc.tensor.matmul(
                            gp[:], wqk_sb[e][:, g * P:(g + 1) * P],
                            hidsl(e, 512 * j, 512), start=(e == 0),
                            stop=(e == NEC - 1))
                    dst = qc if g == 2 else kc
                    nc.vector.tensor_copy(dst[0][j][64:128, :], gp[0:64, :])
                    nc.vector.tensor_copy(dst[1][j][64:128, :], gp[64:128, :])
                # v chunk j
                for st in range(4):
                    vp = gps_pool.tile([P, 256], f32, name=f"vps{j}_{st}",
                                       tag="gps")
                    for e in range(NEC):
                        nc.tensor.matmul(
                            vp[:], hidsl(e, 512 * j + st * P, P),
                            wv_sb[e][:], start=(e == 0), stop=(e == NEC - 1))
                    nc.vector.tensor_copy(vc[0][j][:, st * P:(st + 1) * P],
                                          vp[:, 0:P])
                    nc.vector.tensor_copy(vc[1][j][:, st * P:(st + 1) * P],
                                          vp[:, P:2 * P])
                # rope chunk j
                t0 = rtmp_pool.tile([P, 512], bt, name=f"t0_{j}", tag="t0")
                t1 = rtmp_pool.tile([P, 512], bt, name=f"t1_{j}", tag="t1")
                ta = rtmp_pool.tile([P, 512], bt, name=f"ta_{j}", tag="ta")
                tb = rtmp_pool.tile([P, 512], bt, name=f"tb_{j}", tag="tb")
                nc.vector.tensor_mul(t0[:], g01[0][:], cos_sb[:, qs])
                nc.vector.tensor_mul(t1[:], g01[1][:], sin_sb[:, qs])
                nc.vector.tensor_sub(ta[:], t0[:], t1[:])      # tops
                nc.vector.tensor_mul(t0[:], g01[1][:], cos_sb[:, qs])
                nc.vector.tensor_mul(t1[:], g01[0][:], sin_sb[:, qs])
                nc.vector.tensor_add(tb[:], t0[:], t1[:])      # bottoms
                for pc, dst in enumerate((qc[0], qc[1], kc[0], kc[1])):
                    ps = slice(32 * pc, 32 * (pc + 1))
                    nc.vector.tensor_copy(dst[j][0:32, :], ta[ps, :])
                    nc.vector.tensor_copy(dst[j][32:64, :], tb[ps, :])

                if j == 3:
                    emit_oti_load(1)

                # ---- attention chunk j (both heads) ----
                nk = 4 * j + 4
                for hi in range(2):
                    otp = ot_pool.tile([P, 512], f32, name=f"otp{hi}_{j}",
                                       tag="otp")
                    dacc = dac_pool.tile([P, 512], f32, name=f"dac{hi}",
                                         tag=f"dac{hi}")
                    for i in range(nk):
                        mm = i - 4 * j
                        off = 128 * mm if mm > 0 else 0
                        scp = sc_pool.tile([P, 512], f32, name=f"scp{hi}{j}{i}",
                                           tag="scp")
                        nc.tensor.matmul(
                            scp[:, off:], kc[hi][i // 4][:, (i % 4) * P:
                                                         (i % 4 + 1) * P],
                            qc[hi][j][:, off:], start=True, stop=True)
                        pt = pt_pool.tile([P, 512], bt, name=f"pt{hi}{j}{i}",
                                          tag="pt")
                        nc.scalar.activation(
                            pt[:, off:], scp[:, off:],
                            mybir.ActivationFunctionType.Exp, scale=SCALE)
                        if mm >= 0:
                            nc.vector.tensor_mul(
                                pt[:, off:], pt[:, off:],
                                mask_sb[:, mm * 512 + off:(mm + 1) * 512])
                        nc.tensor.matmul(
                            otp[:, off:], vc[hi][i // 4][:, (i % 4) * P:
                                                         (i % 4 + 1) * P],
                            pt[:, off:],
                            start=(i == 0), stop=(i == nk - 1))
                        if i == 0:
                            nc.vector.tensor_copy(dacc[:], pt[:])
                        else:
                            nc.vector.tensor_add(dacc[:, off:], dacc[:, off:],
                                                 pt[:, off:])
                    dacb = dacb_pool.tile([P, 512], bt, name=f"dacb{hi}_{j}",
                                          tag="dacb")
                    nc.vector.tensor_copy(dacb[:], dacc[:])
                    dnp = sc_pool.tile([P, 512], f32, name=f"dnp{hi}_{j}",
                                       tag="scp")
                    nc.tensor.matmul(dnp[:], ones_sb[:], dacb[:],
                                     start=True, stop=True)
                    den_sb = oo_pool.tile([P, 512], f32, name=f"den{hi}_{j}",
                                          tag="den")
                    nc.vector.reciprocal_approx_fast(den_sb[:], dnp[:])
                    otn = oo_pool.tile([P, 512], bt, name=f"otn{hi}_{j}",
                                       tag="otn")
                    nc.vector.tensor_mul(otn[:], otp[:], den_sb[:])
                    nc.sync.dma_start(ag_in[j][hi * P:(hi + 1) * P, :], otn[:])
                nc.gpsimd.collective_compute(
                    "AllGather",
                    mybir.AluOpType.bypass,
                    replica_groups=[list(range(NCORES))],
                    ins=[ag_in[j].opt()],
                    outs=[ag_out[j].opt()],
                )
                if j == 2:
                    emit_oti_load(0)

            emit_oti_load(2)
            emit_oti_load(3)

            # ---- output projection (PE order: after all attention) ----
            for j in range(NQ):
                emit_oproj(j)


def _build_program():
    import concourse.bass as bass  # noqa: F401
    import concourse.mybir as mybir
    import concourse.tile as tile
    from concourse import bacc

    nc = bacc.Bacc("TRN2", target_bir_lowering=False, debug=False,
                   enable_asserts=False, num_devices=NCORES)
    f32 = mybir.dt.float32
    bt = mybir.dt.bfloat16
    hiddenT = nc.dram_tensor("hiddenT", [E, S], bt, kind="ExternalInput").ap()
    wqkT = nc.dram_tensor("wqkT", [E, 512], bt, kind="ExternalInput").ap()
    wvT = nc.dram_tensor("wvT", [E, 256], bt, kind="ExternalInput").ap()
    woutT = nc.dram_tensor("woutT", [P, NEC * 256], bt, kind="ExternalInput").ap()
    cosT = nc.dram_tensor("cosT", [P, S], bt, kind="ExternalInput").ap()
    sinT = nc.dram_tensor("sinT", [P, S], bt, kind="ExternalInput").ap()
    masks = nc.dram_tensor("masks", [P, 4 * 512], bt, kind="ExternalInput").ap()
    outT = nc.dram_tensor("outT", [2 * P, S], f32, kind="ExternalOutput").ap()

    with tile.TileContext(nc) as tc:
        _kernel_body(tc, outT, hiddenT, wqkT, wvT, woutT, cosT, sinT, masks)
    nc.compile()
    return nc


def get_program():
    if "nc" not in _CACHE:
        _CACHE["nc"] = _build_program()
    return _CACHE["nc"]


def _install_ntff_shim():
    """Provide antenv.axon_hooks (missing in this image) so trace=True works."""
    import sys
    import types
    try:
        import antenv.axon_hooks  # noqa: F401
        return
    except ImportError:
        pass
    import antenv
    mod = types.ModuleType("antenv.axon_hooks")
    mod._hook = None

    def set_axon_ntff_profile_hook(h):
        mod._hook = h

    def get_axon_ntff_profile_hook():
        return mod._hook

    mod.set_axon_ntff_profile_hook = set_axon_ntff_profile_hook
    mod.get_axon_ntff_profile_hook = get_axon_ntff_profile_hook
    sys.modules["antenv.axon_hooks"] = mod
    antenv.axon_hooks = mod
    try:
        from trn_agent_boot.trn_boot import _ntff_profile_via_ctypes
        hook = _ntff_profile_via_ctypes("/opt/axon/libaxon_pjrt.so")
        if hook is not None:
            mod._hook = hook
    except Exception:
        pass


def run(inputs, trace=False):
    """Run on the 8 NeuronCores; returns (out [1,S,E], BassKernelResults)."""
    from concourse import bass_utils

    if trace:
        _install_ntff_shim()
    nc = get_program()
    in_maps = _build_in_maps(inputs["hidden_states"], inputs["w_qkv"],
                             inputs["w_out"])
    res = bass_utils.run_bass_kernel_spmd(
        nc, in_maps, core_ids=list(range(NCORES)), trace=trace)
    outT = np.concatenate([res.results[c]["outT"] for c in range(NCORES)],
                          axis=0)  # [E, S]
    out = np.ascontiguousarray(outT.T).reshape(1, S, E).astype(np.float32)
    return out, res


def kernel(hidden_states, w_qkv, w_out):
    out, _ = run({"hidden_states": hidden_states, "w_qkv": w_qkv,
                  "w_out": w_out})
    return out
